# revision 26
# baseline (speedup 1.0000x reference)
"""Trainium2 Bass kernel for the LGP-instruction module (read -> op bank -> write).

Data-parallel over batch: core b computes x[b] (2048, 4096).
Memory-bound problem: all HBM traffic is 8-bit (x fp8e4m3, out fp8e5m2 as a
scaled delta from a per-chunk mean; params bf16).  rel-err budget (2e-2)
covers this comfortably; most input quantization error is damped because the
output norm is dominated by a constant (sigmoid op) component.

Per T-chunk (tapered sizes to shorten pipeline ramp and tail):
  phase 1: values[C, Tc] = sum_vt rw[vt].T @ x_tile[vt]   (fp8 DoubleRow MMs,
           K=256 per MM; rw scaled by S_r into fp8 range, undone in the
           PSUM->SBUF drain)
  phase 2: h_k = W_k.T @ vals -> ACT f_k(h+b_k) -> DVE bf16 accumulate
           (identity and neg ops pre-merged on host: 7 effective ops);
           then subtract the chunk mean (returned to host via `asum`)
  phase 3: out[Tc, V] = dacc.T @ (wwT*S) -> PSUM f32 -> fp8e5m2 drains
           (split DVE/ACT) -> SWDGE stores.  Host adds mean.T @ wwT back.

Emission interleaves next-chunk phase-1 MMs with current-chunk write pairs so
PSUM drains flow continuously and the PE stays HAM-warm.
"""
import sys
import numpy as np

if '/opt/trn_rl_repo' not in sys.path:
    sys.path.insert(0, '/opt/trn_rl_repo')

B, T, V, C, NOPS = 8, 2048, 4096, 128, 8
NCORES = 8
NV = V // 128                      # 32 v-tiles
CHUNKS = [256, 512, 512, 512, 256]
NCH = len(CHUNKS)
assert sum(CHUNKS) == T
NK = 7                             # effective ops after identity+neg merge
ACT_SET = {1, 3, 5, 7, 9, 11, 13, 15}   # 8/16 psum drain pairs go to ACT

_CACHE = {}
LAST_RESULT = None


def _build(post, inv_sr, inv_so):
    from concourse import bass, bacc, tile, mybir
    f32, bf16 = mybir.dt.float32, mybir.dt.bfloat16
    fp8 = mybir.dt.float8e4
    fp8o = mybir.dt.float8e5
    AF = mybir.ActivationFunctionType
    ts = bass.ts
    FUNCS = [AF.Identity, AF.Relu, AF.Gelu, AF.Square,
             AF.Abs, AF.Tanh, AF.Sigmoid]

    nc = bacc.Bacc("TRN2", target_bir_lowering=False, debug=False,
                   num_devices=NCORES)
    xh = nc.dram_tensor("xh", [128, NV * T], fp8, kind="ExternalInput")
    rw = nc.dram_tensor("rw", [128, NV * C], fp8, kind="ExternalInput")
    wwT = nc.dram_tensor("wwT", [C, V], fp8o, kind="ExternalInput")
    opw = nc.dram_tensor("opw", [C, NK * C], fp8, kind="ExternalInput")
    opb = nc.dram_tensor("opb", [C, NK], f32, kind="ExternalInput")
    out = nc.dram_tensor("out", [T, V], fp8o, kind="ExternalOutput")
    asum = nc.dram_tensor("asum", [C, NCH], f32, kind="ExternalOutput")
    out_r = out.ap().rearrange("(r p) v -> p r v", p=128)

    # chunk c covers t in [toff[c], toff[c]+Tc); x for chunk c is a
    # contiguous [128, NV * Tc] slab at element offset NV * toff[c].
    toff = [sum(CHUNKS[:i]) for i in range(NCH)]

    with tile.TileContext(nc) as tc:
        with tc.tile_pool(name="const", bufs=1) as constp, \
             tc.tile_pool(name="xt", bufs=8) as xtp, \
             tc.tile_pool(name="vals_ps", bufs=1, space="PSUM") as vpsp, \
             tc.tile_pool(name="vals_sb", bufs=2) as vsbp, \
             tc.tile_pool(name="h_ps", bufs=1, space="PSUM") as hpsp, \
             tc.tile_pool(name="t_sb", bufs=3) as tp, \
             tc.tile_pool(name="acc", bufs=2) as accp, \
             tc.tile_pool(name="dacc", bufs=2) as daccp, \
             tc.tile_pool(name="mean", bufs=2) as meanp, \
             tc.tile_pool(name="out_ps", bufs=3, space="PSUM") as opsp, \
             tc.tile_pool(name="out_sb", bufs=4) as osbp:

            # rw piece 0 first on the SP ring (first MMs need it); other
            # consts go on the ACT HWDGE ring so x loads aren't behind them.
            rw_t = constp.tile([128, NV, C], fp8)
            nc.sync.dma_start(rw_t[:, ts(0, NV // 4), :],
                              rw.ap()[:, ts(0, NV * C // 4)])
            opb_t = constp.tile([C, NK], f32)
            nc.gpsimd.dma_start(opb_t[:], opb.ap())
            opw_t = constp.tile([C, NK * C], fp8)
            nc.gpsimd.dma_start(opw_t[:], opw.ap())
            wwT_t = constp.tile([C, V], fp8o)
            nc.gpsimd.dma_start(wwT_t[:], wwT.ap())

            asum_t = constp.tile([C, NCH], f32)
            vals_sb = [None] * NCH
            dacc_sb = [None] * NCH

            def phase1(cn):
                # read: accumulate all v-tiles into one psum bank via fp8
                # DoubleRow (2 v-tiles per MM).  Yields after each MM.
                Tc = CHUNKS[cn]
                values = vpsp.tile([128, Tc], f32)
                if cn == 0:
                    blocks = [(0, 4), (4, 4), (8, 8), (16, 16)]
                else:
                    blocks = [(0, 16), (16, 16)]
                for bi, (v0, nvb) in enumerate(blocks):
                    xt = xtp.tile([128, nvb, Tc], fp8)
                    start_el = NV * toff[cn] + v0 * Tc
                    assert start_el % (nvb * Tc) == 0
                    nc.sync.dma_start(
                        xt[:], xh.ap()[:, ts(start_el // (nvb * Tc),
                                             nvb * Tc)])
                    if cn == 0 and bi in (1, 3):
                        # rw piece 1 after the leading x pieces; pieces 2-3
                        # after the last chunk-0 block (their MMs run later)
                        for q in ((1,) if bi == 1 else (2, 3)):
                            nc.sync.dma_start(
                                rw_t[:, ts(q, NV // 4), :],
                                rw.ap()[:, ts(q, NV * C // 4)])
                    for j in range(0, nvb, 2):
                        vt = v0 + j
                        nc.tensor.matmul(values[:], rw_t[:, ts(vt // 2, 2), :],
                                         xt[:, ts(j // 2, 2), :],
                                         start=(vt == 0), stop=(vt == NV - 2),
                                         perf_mode=mybir.MatmulPerfMode.DoubleRow)
                        yield
                vals = vsbp.tile([128, Tc], bf16)
                nc.vector.tensor_scalar_mul(vals[:], values[:], inv_sr)
                vals_sb[cn] = vals

            def opbank(cn):
                # generator: yields after each op's act so chain steps can be
                # woven between the previous chunk's psum drains
                Tc = CHUNKS[cn]
                vals = vals_sb[cn]
                acc = accp.tile([128, Tc], bf16)
                for k in range(NK):
                    h = hpsp.tile([128, Tc], f32)
                    nc.tensor.matmul(h[:], opw_t[:, ts(k, C)], vals[:],
                                     start=True, stop=True)
                    if k == 0:
                        nc.scalar.activation(acc[:], h[:], FUNCS[0],
                                             bias=opb_t[:, 0:1], scale=inv_so)
                    else:
                        t = tp.tile([128, Tc], bf16)
                        nc.scalar.activation(t[:], h[:], FUNCS[k],
                                             bias=opb_t[:, k:k + 1],
                                             scale=inv_so)
                        nc.vector.scalar_tensor_tensor(
                            acc[:], t[:], post[k], acc[:],
                            op0=mybir.AluOpType.mult, op1=mybir.AluOpType.add)
                    yield
                # subtract the chunk mean (negated sum returned via asum);
                # host adds (-asum/Tc) @ wwT back.
                nc.vector.tensor_reduce(asum_t[:, cn:cn + 1], acc[:],
                                        axis=mybir.AxisListType.X,
                                        op=mybir.AluOpType.add, negate=True)
                negmean = meanp.tile([C, 1], f32)
                nc.vector.tensor_scalar_mul(negmean[:], asum_t[:, cn:cn + 1],
                                            1.0 / Tc)
                dacc = daccp.tile([128, Tc], bf16)
                nc.vector.tensor_scalar_add(dacc[:], acc[:], negmean[:])
                dacc_sb[cn] = dacc

            def write(cn, dacc):
                Tc = CHUNKS[cn]
                nsub = Tc // 128
                row0 = toff[cn] // 128
                for sub2 in range(nsub // 2):
                    osb = osbp.tile([128, 2, V], fp8o)
                    for s in range(2):
                        sub = sub2 * 2 + s
                        for nn2 in range(4):
                            ops2 = opsp.tile([128, 1024], f32)
                            nc.tensor.matmul(ops2[:, 0:512],
                                             dacc[:, ts(sub, 128)],
                                             wwT_t[:, ts(nn2 * 2, 512)],
                                             start=True, stop=True)
                            nc.tensor.matmul(ops2[:, 512:1024],
                                             dacc[:, ts(sub, 128)],
                                             wwT_t[:, ts(nn2 * 2 + 1, 512)],
                                             start=True, stop=True)
                            if (sub * 4 + nn2) in ACT_SET:
                                nc.scalar.copy(osb[:, s, ts(nn2, 1024)],
                                               ops2[:])
                            else:
                                nc.vector.tensor_copy(osb[:, s, ts(nn2, 1024)],
                                                      ops2[:])
                            yield
                    nc.gpsimd.dma_start(
                        out_r[:, ts(row0 // 2 + sub2, 2), :], osb[:])

            def drain(g):
                if g is not None:
                    for _ in g:
                        pass

            def step(g, n):
                if g is None:
                    return None
                for _ in range(n):
                    if next(g, 'done') == 'done':
                        return None
                return g

            # prologue: phase1(0), then weave opbank(0) with phase1(1)
            for _ in phase1(0):
                pass
            g_ob = iter(opbank(0))
            g_p1 = iter(phase1(1))
            while g_ob is not None or g_p1 is not None:
                g_ob = step(g_ob, 1)
                g_p1 = step(g_p1, 2)
            # steady state: write(c) woven with phase1(c+2) and opbank(c+1)
            for c in range(NCH):
                g_p1 = iter(phase1(c + 2)) if c + 2 < NCH else None
                g_ob = iter(opbank(c + 1)) if c + 1 < NCH else None
                if c == NCH - 1:
                    nc.gpsimd.dma_start(asum.ap(), asum_t[:])
                nw = (CHUNKS[c] // 128) * 4
                r1 = max(1, (16 + nw - 1) // nw)
                for _ in write(c, dacc_sb[c]):
                    g_p1 = step(g_p1, r1)
                    g_ob = step(g_ob, 1)
                drain(g_p1)
                drain(g_ob)
    nc.compile()
    return nc


def _softmax(x, axis):
    x = np.asarray(x, np.float32)
    m = x.max(axis=axis, keepdims=True)
    e = np.exp(x - m)
    return e / e.sum(axis=axis, keepdims=True)


def kernel(x, basis, read_coeffs, write_coeffs, op_logits, op_weights,
           op_biases, out_scale):
    global LAST_RESULT
    import ml_dtypes
    from concourse.bass_utils import run_bass_kernel_spmd
    bf16 = ml_dtypes.bfloat16

    x = np.asarray(x, np.float32)
    basis = np.asarray(basis, np.float32)
    read_coeffs = np.asarray(read_coeffs, np.float32)
    write_coeffs = np.asarray(write_coeffs, np.float32)
    op_logits = np.asarray(op_logits, np.float32)
    op_weights = np.asarray(op_weights, np.float64)
    op_biases = np.asarray(op_biases, np.float64)
    out_scale = np.float32(out_scale)

    read_w = _softmax(basis @ read_coeffs.T, axis=0)               # (V, C)
    wwT = np.ascontiguousarray((basis @ write_coeffs.T).T) * out_scale  # (C, V)
    w = _softmax(op_logits, axis=0).astype(np.float64)

    # Fold mixture weights into op weights/biases where the nonlinearity
    # allows; merge the two linear ops (identity, neg) into one.
    #   orig i: 0 ident, 1 relu, 2 gelu, 3 square, 4 neg, 5 abs, 6 tanh, 7 sigm
    Wm = [w[0] * op_weights[0] - w[4] * op_weights[4],
          w[1] * op_weights[1],
          op_weights[2],
          np.sqrt(w[3]) * op_weights[3],
          w[5] * op_weights[5],
          op_weights[6],
          op_weights[7]]
    bm = [w[0] * op_biases[0] - w[4] * op_biases[4],
          w[1] * op_biases[1],
          op_biases[2],
          np.sqrt(w[3]) * op_biases[3],
          w[5] * op_biases[5],
          op_biases[6],
          op_biases[7]]
    post = [1.0, 1.0, float(w[2]), 1.0, 1.0, float(w[6]), float(w[7])]

    S_r = float(2 ** int(np.floor(np.log2(300.0 / read_w.max()))))
    S_o = 64.0
    key = tuple(post) + (float(w[0]), float(w[4]), S_r, S_o)
    if key not in _CACHE:
        _CACHE[key] = _build(post, 1.0 / S_r, 1.0 / S_o)
    nc = _CACHE[key]

    opw_eff = np.stack(Wm).astype(np.float32)          # (NK, C, C)
    opb_eff = np.stack(bm).astype(np.float32).T        # (C, NK)

    # rw: (V, C) -> [p, vt, c];  opw: (NK, C, C) -> [p, k, c]
    rwH = np.ascontiguousarray(
        read_w.reshape(NV, 128, C).transpose(1, 0, 2)).reshape(128, NV * C)
    opwH = np.ascontiguousarray(
        opw_eff.transpose(1, 0, 2)).reshape(C, NK * C)

    S = 1024.0
    shared = {
        "rw": (rwH * S_r).astype(ml_dtypes.float8_e4m3),
        "wwT": (wwT * S).astype(ml_dtypes.float8_e5m2),
        "opw": (opwH * S_o).astype(ml_dtypes.float8_e4m3),
        "opb": np.ascontiguousarray(opb_eff),
    }
    # x[b] (T, V): per chunk c a [p, vt, t] slab, concatenated
    x8 = x.astype(ml_dtypes.float8_e4m3)
    toff = [sum(CHUNKS[:i]) for i in range(NCH)]
    in_maps = []
    for b in range(B):
        slabs = []
        for c, Tc in enumerate(CHUNKS):
            xc = x8[b][toff[c]:toff[c] + Tc]           # (Tc, V)
            # (Tc, V) -> [p, vt, t] with v = vt*128 + p
            slabs.append(np.ascontiguousarray(
                xc.reshape(Tc, NV, 128).transpose(2, 1, 0)).reshape(128, -1))
        m = dict(shared)
        m["xh"] = np.ascontiguousarray(np.concatenate(slabs, axis=1))
        in_maps.append(m)

    res = run_bass_kernel_spmd(nc, in_maps, core_ids=list(range(NCORES)))
    LAST_RESULT = res
    out = np.empty((B, T, V), np.float32)
    for b in range(B):
        d = np.asarray(res.results[b]["out"], np.float32) / S
        negsum = np.asarray(res.results[b]["asum"], np.float32)  # (C, NCH)
        for c, Tc in enumerate(CHUNKS):
            mean = -negsum[:, c] / Tc                            # (C,)
            base = mean @ wwT                                    # (V,)
            out[b, toff[c]:toff[c] + Tc] = d[toff[c]:toff[c] + Tc] + base
    return out


# revision 27
# speedup vs baseline: 1.0073x; 1.0073x over previous
"""Trainium2 Bass kernel for the LGP-instruction module (read -> op bank -> write).

Data-parallel over batch: core b computes x[b] (2048, 4096).
Memory-bound problem: all HBM traffic is 8-bit (x fp8e4m3, out fp8e5m2 as a
scaled delta from a per-chunk mean; params bf16).  rel-err budget (2e-2)
covers this comfortably; most input quantization error is damped because the
output norm is dominated by a constant (sigmoid op) component.

Per T-chunk (tapered sizes to shorten pipeline ramp and tail):
  phase 1: values[C, Tc] = sum_vt rw[vt].T @ x_tile[vt]   (fp8 DoubleRow MMs,
           K=256 per MM; rw scaled by S_r into fp8 range, undone in the
           PSUM->SBUF drain)
  phase 2: h_k = W_k.T @ vals -> ACT f_k(h+b_k) -> DVE bf16 accumulate
           (identity and neg ops pre-merged on host: 7 effective ops);
           then subtract the chunk mean (returned to host via `asum`)
  phase 3: out[Tc, V] = dacc.T @ (wwT*S) -> PSUM f32 -> fp8e5m2 drains
           (split DVE/ACT) -> SWDGE stores.  Host adds mean.T @ wwT back.

Emission interleaves next-chunk phase-1 MMs with current-chunk write pairs so
PSUM drains flow continuously and the PE stays HAM-warm.
"""
import sys
import numpy as np

if '/opt/trn_rl_repo' not in sys.path:
    sys.path.insert(0, '/opt/trn_rl_repo')

B, T, V, C, NOPS = 8, 2048, 4096, 128, 8
NCORES = 8
NV = V // 128                      # 32 v-tiles
CHUNKS = [256, 512, 512, 512, 256]
NCH = len(CHUNKS)
assert sum(CHUNKS) == T
NK = 7                             # effective ops after identity+neg merge
ACT_SET = {1, 3, 5, 7, 9, 11, 13, 15}   # 8/16 psum drain pairs go to ACT

_CACHE = {}
LAST_RESULT = None


def _build(post, inv_sr, inv_so):
    from concourse import bass, bacc, tile, mybir
    f32, bf16 = mybir.dt.float32, mybir.dt.bfloat16
    fp8 = mybir.dt.float8e4
    fp8o = mybir.dt.float8e5
    AF = mybir.ActivationFunctionType
    ts = bass.ts
    FUNCS = [AF.Identity, AF.Relu, AF.Gelu, AF.Square,
             AF.Abs, AF.Tanh, AF.Sigmoid]

    nc = bacc.Bacc("TRN2", target_bir_lowering=False, debug=False,
                   num_devices=NCORES)
    xh = nc.dram_tensor("xh", [128, NV * T], fp8, kind="ExternalInput")
    rw = nc.dram_tensor("rw", [128, NV * C], fp8, kind="ExternalInput")
    wwT = nc.dram_tensor("wwT", [C, V], fp8o, kind="ExternalInput")
    opw = nc.dram_tensor("opw", [C, NK * C], fp8, kind="ExternalInput")
    opb = nc.dram_tensor("opb", [C, NK], f32, kind="ExternalInput")
    out = nc.dram_tensor("out", [T, V], fp8o, kind="ExternalOutput")
    asum = nc.dram_tensor("asum", [C, NCH], f32, kind="ExternalOutput")
    out_r = out.ap().rearrange("(r p) v -> p r v", p=128)

    # chunk c covers t in [toff[c], toff[c]+Tc); x for chunk c is a
    # contiguous [128, NV * Tc] slab at element offset NV * toff[c].
    toff = [sum(CHUNKS[:i]) for i in range(NCH)]

    with tile.TileContext(nc) as tc:
        with tc.tile_pool(name="const", bufs=1) as constp, \
             tc.tile_pool(name="xt", bufs=6) as xtp, \
             tc.tile_pool(name="vals_ps", bufs=1, space="PSUM") as vpsp, \
             tc.tile_pool(name="vals_sb", bufs=2) as vsbp, \
             tc.tile_pool(name="h_ps", bufs=1, space="PSUM") as hpsp, \
             tc.tile_pool(name="t_sb", bufs=3) as tp, \
             tc.tile_pool(name="acc", bufs=2) as accp, \
             tc.tile_pool(name="dacc", bufs=2) as daccp, \
             tc.tile_pool(name="mean", bufs=2) as meanp, \
             tc.tile_pool(name="out_ps", bufs=3, space="PSUM") as opsp, \
             tc.tile_pool(name="out_sb", bufs=4) as osbp:

            # rw piece 0 first on the SP ring (first MMs need it); other
            # consts go on the ACT HWDGE ring so x loads aren't behind them.
            rw_t = constp.tile([128, NV, C], fp8)
            nc.sync.dma_start(rw_t[:, ts(0, NV // 4), :],
                              rw.ap()[:, ts(0, NV * C // 4)])
            opb_t = constp.tile([C, NK], f32)
            nc.gpsimd.dma_start(opb_t[:], opb.ap())
            opw_t = constp.tile([C, NK * C], fp8)
            nc.gpsimd.dma_start(opw_t[:], opw.ap())
            wwT_t = constp.tile([C, V], fp8o)
            nc.gpsimd.dma_start(wwT_t[:], wwT.ap())

            asum_t = constp.tile([C, NCH], f32)
            vals_sb = [None] * NCH
            dacc_sb = [None] * NCH

            def phase1(cn):
                # read: accumulate all v-tiles into one psum bank via fp8
                # DoubleRow (2 v-tiles per MM).  Yields after each MM.
                Tc = CHUNKS[cn]
                values = vpsp.tile([128, Tc], f32)
                if cn == 0:
                    blocks = [(0, 4), (4, 4), (8, 8), (16, 16)]
                else:
                    blocks = [(0, 16), (16, 16)]
                for bi, (v0, nvb) in enumerate(blocks):
                    xt = xtp.tile([128, nvb, Tc], fp8)
                    start_el = NV * toff[cn] + v0 * Tc
                    assert start_el % (nvb * Tc) == 0
                    nc.sync.dma_start(
                        xt[:], xh.ap()[:, ts(start_el // (nvb * Tc),
                                             nvb * Tc)])
                    if cn == 0 and bi in (1, 3):
                        # rw piece 1 after the leading x pieces; pieces 2-3
                        # after the last chunk-0 block (their MMs run later)
                        for q in ((1,) if bi == 1 else (2, 3)):
                            nc.sync.dma_start(
                                rw_t[:, ts(q, NV // 4), :],
                                rw.ap()[:, ts(q, NV * C // 4)])
                    for j in range(0, nvb, 2):
                        vt = v0 + j
                        nc.tensor.matmul(values[:], rw_t[:, ts(vt // 2, 2), :],
                                         xt[:, ts(j // 2, 2), :],
                                         start=(vt == 0), stop=(vt == NV - 2),
                                         perf_mode=mybir.MatmulPerfMode.DoubleRow)
                        yield
                vals = vsbp.tile([128, Tc], bf16)
                nc.vector.tensor_scalar_mul(vals[:], values[:], inv_sr)
                vals_sb[cn] = vals

            def opbank(cn):
                # generator: yields after each op's act so chain steps can be
                # woven between the previous chunk's psum drains
                Tc = CHUNKS[cn]
                vals = vals_sb[cn]
                acc = accp.tile([128, Tc], bf16)
                for k in range(NK):
                    h = hpsp.tile([128, Tc], f32)
                    nc.tensor.matmul(h[:], opw_t[:, ts(k, C)], vals[:],
                                     start=True, stop=True)
                    if k == 0:
                        nc.scalar.activation(acc[:], h[:], FUNCS[0],
                                             bias=opb_t[:, 0:1], scale=inv_so)
                    else:
                        t = tp.tile([128, Tc], bf16)
                        nc.scalar.activation(t[:], h[:], FUNCS[k],
                                             bias=opb_t[:, k:k + 1],
                                             scale=inv_so)
                        nc.vector.scalar_tensor_tensor(
                            acc[:], t[:], post[k], acc[:],
                            op0=mybir.AluOpType.mult, op1=mybir.AluOpType.add)
                    yield
                # subtract the chunk mean (negated sum returned via asum);
                # host adds (-asum/Tc) @ wwT back.
                nc.vector.tensor_reduce(asum_t[:, cn:cn + 1], acc[:],
                                        axis=mybir.AxisListType.X,
                                        op=mybir.AluOpType.add, negate=True)
                negmean = meanp.tile([C, 1], f32)
                nc.vector.tensor_scalar_mul(negmean[:], asum_t[:, cn:cn + 1],
                                            1.0 / Tc)
                dacc = daccp.tile([128, Tc], bf16)
                nc.vector.tensor_scalar_add(dacc[:], acc[:], negmean[:])
                dacc_sb[cn] = dacc

            def write(cn, dacc):
                Tc = CHUNKS[cn]
                nsub = Tc // 128
                row0 = toff[cn] // 128
                for sub2 in range(nsub // 2):
                    osb = osbp.tile([128, 2, V], fp8o)
                    for s in range(2):
                        sub = sub2 * 2 + s
                        for nn2 in range(4):
                            ops2 = opsp.tile([128, 1024], f32)
                            nc.tensor.matmul(ops2[:, 0:512],
                                             dacc[:, ts(sub, 128)],
                                             wwT_t[:, ts(nn2 * 2, 512)],
                                             start=True, stop=True)
                            nc.tensor.matmul(ops2[:, 512:1024],
                                             dacc[:, ts(sub, 128)],
                                             wwT_t[:, ts(nn2 * 2 + 1, 512)],
                                             start=True, stop=True)
                            if (sub * 4 + nn2) in ACT_SET:
                                nc.scalar.copy(osb[:, s, ts(nn2, 1024)],
                                               ops2[:])
                            else:
                                nc.vector.tensor_copy(osb[:, s, ts(nn2, 1024)],
                                                      ops2[:])
                            yield
                    nc.gpsimd.dma_start(
                        out_r[:, ts(row0 // 2 + sub2, 2), :], osb[:])

            def drain(g):
                if g is not None:
                    for _ in g:
                        pass

            def step(g, n):
                if g is None:
                    return None
                for _ in range(n):
                    if next(g, 'done') == 'done':
                        return None
                return g

            # prologue: phase1(0), then weave opbank(0) with phase1(1)
            for _ in phase1(0):
                pass
            g_ob = iter(opbank(0))
            g_p1 = iter(phase1(1))
            while g_ob is not None or g_p1 is not None:
                g_ob = step(g_ob, 1)
                g_p1 = step(g_p1, 2)
            # steady state: write(c) woven with phase1(c+2) and opbank(c+1)
            for c in range(NCH):
                g_p1 = iter(phase1(c + 2)) if c + 2 < NCH else None
                g_ob = iter(opbank(c + 1)) if c + 1 < NCH else None
                if c == NCH - 1:
                    nc.gpsimd.dma_start(asum.ap(), asum_t[:])
                nw = (CHUNKS[c] // 128) * 4
                r1 = max(1, (16 + nw - 1) // nw)
                for _ in write(c, dacc_sb[c]):
                    g_p1 = step(g_p1, r1)
                    g_ob = step(g_ob, 1)
                drain(g_p1)
                drain(g_ob)
    nc.compile()
    return nc


def _softmax(x, axis):
    x = np.asarray(x, np.float32)
    m = x.max(axis=axis, keepdims=True)
    e = np.exp(x - m)
    return e / e.sum(axis=axis, keepdims=True)


def kernel(x, basis, read_coeffs, write_coeffs, op_logits, op_weights,
           op_biases, out_scale):
    global LAST_RESULT
    import ml_dtypes
    from concourse.bass_utils import run_bass_kernel_spmd
    bf16 = ml_dtypes.bfloat16

    x = np.asarray(x, np.float32)
    basis = np.asarray(basis, np.float32)
    read_coeffs = np.asarray(read_coeffs, np.float32)
    write_coeffs = np.asarray(write_coeffs, np.float32)
    op_logits = np.asarray(op_logits, np.float32)
    op_weights = np.asarray(op_weights, np.float64)
    op_biases = np.asarray(op_biases, np.float64)
    out_scale = np.float32(out_scale)

    read_w = _softmax(basis @ read_coeffs.T, axis=0)               # (V, C)
    wwT = np.ascontiguousarray((basis @ write_coeffs.T).T) * out_scale  # (C, V)
    w = _softmax(op_logits, axis=0).astype(np.float64)

    # Fold mixture weights into op weights/biases where the nonlinearity
    # allows; merge the two linear ops (identity, neg) into one.
    #   orig i: 0 ident, 1 relu, 2 gelu, 3 square, 4 neg, 5 abs, 6 tanh, 7 sigm
    Wm = [w[0] * op_weights[0] - w[4] * op_weights[4],
          w[1] * op_weights[1],
          op_weights[2],
          np.sqrt(w[3]) * op_weights[3],
          w[5] * op_weights[5],
          op_weights[6],
          op_weights[7]]
    bm = [w[0] * op_biases[0] - w[4] * op_biases[4],
          w[1] * op_biases[1],
          op_biases[2],
          np.sqrt(w[3]) * op_biases[3],
          w[5] * op_biases[5],
          op_biases[6],
          op_biases[7]]
    post = [1.0, 1.0, float(w[2]), 1.0, 1.0, float(w[6]), float(w[7])]

    S_r = float(2 ** int(np.floor(np.log2(300.0 / read_w.max()))))
    S_o = 64.0
    key = tuple(post) + (float(w[0]), float(w[4]), S_r, S_o)
    if key not in _CACHE:
        _CACHE[key] = _build(post, 1.0 / S_r, 1.0 / S_o)
    nc = _CACHE[key]

    opw_eff = np.stack(Wm).astype(np.float32)          # (NK, C, C)
    opb_eff = np.stack(bm).astype(np.float32).T        # (C, NK)

    # rw: (V, C) -> [p, vt, c];  opw: (NK, C, C) -> [p, k, c]
    rwH = np.ascontiguousarray(
        read_w.reshape(NV, 128, C).transpose(1, 0, 2)).reshape(128, NV * C)
    opwH = np.ascontiguousarray(
        opw_eff.transpose(1, 0, 2)).reshape(C, NK * C)

    S = 1024.0
    shared = {
        "rw": (rwH * S_r).astype(ml_dtypes.float8_e4m3),
        "wwT": (wwT * S).astype(ml_dtypes.float8_e5m2),
        "opw": (opwH * S_o).astype(ml_dtypes.float8_e4m3),
        "opb": np.ascontiguousarray(opb_eff),
    }
    # x[b] (T, V): per chunk c a [p, vt, t] slab, concatenated
    x8 = x.astype(ml_dtypes.float8_e4m3)
    toff = [sum(CHUNKS[:i]) for i in range(NCH)]
    in_maps = []
    for b in range(B):
        slabs = []
        for c, Tc in enumerate(CHUNKS):
            xc = x8[b][toff[c]:toff[c] + Tc]           # (Tc, V)
            # (Tc, V) -> [p, vt, t] with v = vt*128 + p
            slabs.append(np.ascontiguousarray(
                xc.reshape(Tc, NV, 128).transpose(2, 1, 0)).reshape(128, -1))
        m = dict(shared)
        m["xh"] = np.ascontiguousarray(np.concatenate(slabs, axis=1))
        in_maps.append(m)

    res = run_bass_kernel_spmd(nc, in_maps, core_ids=list(range(NCORES)))
    LAST_RESULT = res
    out = np.empty((B, T, V), np.float32)
    for b in range(B):
        d = np.asarray(res.results[b]["out"], np.float32) / S
        negsum = np.asarray(res.results[b]["asum"], np.float32)  # (C, NCH)
        for c, Tc in enumerate(CHUNKS):
            mean = -negsum[:, c] / Tc                            # (C,)
            base = mean @ wwT                                    # (V,)
            out[b, toff[c]:toff[c] + Tc] = d[toff[c]:toff[c] + Tc] + base
    return out


# revision 28
# speedup vs baseline: 1.0193x; 1.0120x over previous
"""Trainium2 Bass kernel for the LGP-instruction module (read -> op bank -> write).

Data-parallel over batch: core b computes x[b] (2048, 4096).
Memory-bound problem: all HBM traffic is 8-bit (x fp8e4m3, out fp8e5m2 as a
scaled delta from a per-chunk mean; params bf16).  rel-err budget (2e-2)
covers this comfortably; most input quantization error is damped because the
output norm is dominated by a constant (sigmoid op) component.

Per T-chunk (tapered sizes to shorten pipeline ramp and tail):
  phase 1: values[C, Tc] = sum_vt rw[vt].T @ x_tile[vt]   (fp8 DoubleRow MMs,
           K=256 per MM; rw scaled by S_r into fp8 range, undone in the
           PSUM->SBUF drain)
  phase 2: h_k = W_k.T @ vals -> ACT f_k(h+b_k) -> DVE bf16 accumulate
           (identity and neg ops pre-merged on host: 7 effective ops);
           then subtract the chunk mean (returned to host via `asum`)
  phase 3: out[Tc, V] = dacc.T @ (wwT*S) -> PSUM f32 -> fp8e5m2 drains
           (split DVE/ACT) -> SWDGE stores.  Host adds mean.T @ wwT back.

Emission interleaves next-chunk phase-1 MMs with current-chunk write pairs so
PSUM drains flow continuously and the PE stays HAM-warm.
"""
import sys
import numpy as np

if '/opt/trn_rl_repo' not in sys.path:
    sys.path.insert(0, '/opt/trn_rl_repo')

B, T, V, C, NOPS = 8, 2048, 4096, 128, 8
NCORES = 8
NV = V // 128                      # 32 v-tiles
CHUNKS = [256, 512, 512, 512, 256]
NCH = len(CHUNKS)
assert sum(CHUNKS) == T
NK = 7                             # effective ops after identity+neg merge
ACT_SET = {1, 3, 5, 7, 9, 11, 13, 15}   # 8/16 psum drain pairs go to ACT

_CACHE = {}
LAST_RESULT = None


def _build(post, inv_sr, inv_so):
    from concourse import bass, bacc, tile, mybir
    f32, bf16 = mybir.dt.float32, mybir.dt.bfloat16
    fp8 = mybir.dt.float8e4
    fp8o = mybir.dt.float8e5
    AF = mybir.ActivationFunctionType
    ts = bass.ts
    FUNCS = [AF.Identity, AF.Relu, AF.Gelu, AF.Square,
             AF.Abs, AF.Tanh, AF.Sigmoid]

    nc = bacc.Bacc("TRN2", target_bir_lowering=False, debug=False,
                   num_devices=NCORES)
    xh = nc.dram_tensor("xh", [128, NV * T], fp8, kind="ExternalInput")
    rw = nc.dram_tensor("rw", [128, NV * C], fp8, kind="ExternalInput")
    wwT = nc.dram_tensor("wwT", [C, V], fp8o, kind="ExternalInput")
    opw = nc.dram_tensor("opw", [C, NK * C], fp8, kind="ExternalInput")
    opb = nc.dram_tensor("opb", [C, NK], f32, kind="ExternalInput")
    out = nc.dram_tensor("out", [T, V], fp8o, kind="ExternalOutput")
    asum = nc.dram_tensor("asum", [C, NCH], f32, kind="ExternalOutput")
    out_r = out.ap().rearrange("(r p) v -> p r v", p=128)

    # chunk c covers t in [toff[c], toff[c]+Tc); x for chunk c is a
    # contiguous [128, NV * Tc] slab at element offset NV * toff[c].
    toff = [sum(CHUNKS[:i]) for i in range(NCH)]

    with tile.TileContext(nc) as tc:
        with tc.tile_pool(name="const", bufs=1) as constp, \
             tc.tile_pool(name="xt", bufs=6) as xtp, \
             tc.tile_pool(name="vals_ps", bufs=1, space="PSUM") as vpsp, \
             tc.tile_pool(name="vals_sb", bufs=2) as vsbp, \
             tc.tile_pool(name="h_ps", bufs=1, space="PSUM") as hpsp, \
             tc.tile_pool(name="t_sb", bufs=3) as tp, \
             tc.tile_pool(name="acc", bufs=2) as accp, \
             tc.tile_pool(name="dacc", bufs=2) as daccp, \
             tc.tile_pool(name="mean", bufs=2) as meanp, \
             tc.tile_pool(name="out_ps", bufs=3, space="PSUM") as opsp, \
             tc.tile_pool(name="out_sb", bufs=4) as osbp:

            # rw piece 0 first on the SP ring (first MMs need it); other
            # consts go on the ACT HWDGE ring so x loads aren't behind them.
            rw_t = constp.tile([128, NV, C], fp8)
            nc.sync.dma_start(rw_t[:, ts(0, NV // 4), :],
                              rw.ap()[:, ts(0, NV * C // 4)])
            opb_t = constp.tile([C, NK], f32)
            nc.scalar.dma_start(opb_t[:], opb.ap())
            opw_t = constp.tile([C, NK * C], fp8)
            nc.scalar.dma_start(opw_t[:], opw.ap())
            wwT_t = constp.tile([C, V], fp8o)
            nc.scalar.dma_start(wwT_t[:], wwT.ap())

            asum_t = constp.tile([C, NCH], f32)
            vals_sb = [None] * NCH
            dacc_sb = [None] * NCH

            def phase1(cn):
                # read: accumulate all v-tiles into one psum bank via fp8
                # DoubleRow (2 v-tiles per MM).  Yields after each MM.
                Tc = CHUNKS[cn]
                values = vpsp.tile([128, Tc], f32)
                if cn == 0:
                    blocks = [(0, 4), (4, 4), (8, 8), (16, 16)]
                else:
                    blocks = [(0, 16), (16, 16)]
                for bi, (v0, nvb) in enumerate(blocks):
                    xt = xtp.tile([128, nvb, Tc], fp8)
                    start_el = NV * toff[cn] + v0 * Tc
                    assert start_el % (nvb * Tc) == 0
                    nc.sync.dma_start(
                        xt[:], xh.ap()[:, ts(start_el // (nvb * Tc),
                                             nvb * Tc)])
                    if cn == 0 and bi in (1, 3):
                        # rw piece 1 after the leading x pieces; pieces 2-3
                        # after the last chunk-0 block (their MMs run later)
                        for q in ((1,) if bi == 1 else (2, 3)):
                            nc.sync.dma_start(
                                rw_t[:, ts(q, NV // 4), :],
                                rw.ap()[:, ts(q, NV * C // 4)])
                    for j in range(0, nvb, 2):
                        vt = v0 + j
                        nc.tensor.matmul(values[:], rw_t[:, ts(vt // 2, 2), :],
                                         xt[:, ts(j // 2, 2), :],
                                         start=(vt == 0), stop=(vt == NV - 2),
                                         perf_mode=mybir.MatmulPerfMode.DoubleRow)
                        yield
                vals = vsbp.tile([128, Tc], bf16)
                nc.vector.tensor_scalar_mul(vals[:], values[:], inv_sr)
                vals_sb[cn] = vals

            def opbank(cn):
                # generator: yields after each op's act so chain steps can be
                # woven between the previous chunk's psum drains
                Tc = CHUNKS[cn]
                vals = vals_sb[cn]
                acc = accp.tile([128, Tc], bf16)
                for k in range(NK):
                    h = hpsp.tile([128, Tc], f32)
                    nc.tensor.matmul(h[:], opw_t[:, ts(k, C)], vals[:],
                                     start=True, stop=True)
                    if k == 0:
                        nc.scalar.activation(acc[:], h[:], FUNCS[0],
                                             bias=opb_t[:, 0:1], scale=inv_so)
                    else:
                        t = tp.tile([128, Tc], bf16)
                        nc.scalar.activation(t[:], h[:], FUNCS[k],
                                             bias=opb_t[:, k:k + 1],
                                             scale=inv_so)
                        nc.vector.scalar_tensor_tensor(
                            acc[:], t[:], post[k], acc[:],
                            op0=mybir.AluOpType.mult, op1=mybir.AluOpType.add)
                    yield
                # subtract the chunk mean (negated sum returned via asum);
                # host adds (-asum/Tc) @ wwT back.
                nc.vector.tensor_reduce(asum_t[:, cn:cn + 1], acc[:],
                                        axis=mybir.AxisListType.X,
                                        op=mybir.AluOpType.add, negate=True)
                negmean = meanp.tile([C, 1], f32)
                nc.vector.tensor_scalar_mul(negmean[:], asum_t[:, cn:cn + 1],
                                            1.0 / Tc)
                dacc = daccp.tile([128, Tc], bf16)
                nc.vector.tensor_scalar_add(dacc[:], acc[:], negmean[:])
                dacc_sb[cn] = dacc

            def write(cn, dacc):
                Tc = CHUNKS[cn]
                nsub = Tc // 128
                row0 = toff[cn] // 128
                for sub2 in range(nsub // 2):
                    osb = osbp.tile([128, 2, V], fp8o)
                    for s in range(2):
                        sub = sub2 * 2 + s
                        for nn2 in range(4):
                            ops2 = opsp.tile([128, 1024], f32)
                            nc.tensor.matmul(ops2[:, 0:512],
                                             dacc[:, ts(sub, 128)],
                                             wwT_t[:, ts(nn2 * 2, 512)],
                                             start=True, stop=True)
                            nc.tensor.matmul(ops2[:, 512:1024],
                                             dacc[:, ts(sub, 128)],
                                             wwT_t[:, ts(nn2 * 2 + 1, 512)],
                                             start=True, stop=True)
                            if (sub * 4 + nn2) in ACT_SET:
                                nc.scalar.copy(osb[:, s, ts(nn2, 1024)],
                                               ops2[:])
                            else:
                                nc.vector.tensor_copy(osb[:, s, ts(nn2, 1024)],
                                                      ops2[:])
                            yield
                    nc.gpsimd.dma_start(
                        out_r[:, ts(row0 // 2 + sub2, 2), :], osb[:])

            def drain(g):
                if g is not None:
                    for _ in g:
                        pass

            def step(g, n):
                if g is None:
                    return None
                for _ in range(n):
                    if next(g, 'done') == 'done':
                        return None
                return g

            # prologue: phase1(0), then weave opbank(0) with phase1(1)
            for _ in phase1(0):
                pass
            g_ob = iter(opbank(0))
            g_p1 = iter(phase1(1))
            while g_ob is not None or g_p1 is not None:
                g_ob = step(g_ob, 1)
                g_p1 = step(g_p1, 2)
            # steady state: write(c) woven with phase1(c+2) and opbank(c+1)
            for c in range(NCH):
                g_p1 = iter(phase1(c + 2)) if c + 2 < NCH else None
                g_ob = iter(opbank(c + 1)) if c + 1 < NCH else None
                if c == NCH - 1:
                    nc.gpsimd.dma_start(asum.ap(), asum_t[:])
                nw = (CHUNKS[c] // 128) * 4
                r1 = max(1, (16 + nw - 1) // nw)
                for _ in write(c, dacc_sb[c]):
                    g_p1 = step(g_p1, r1)
                    g_ob = step(g_ob, 1)
                drain(g_p1)
                drain(g_ob)
    nc.compile()
    return nc


def _softmax(x, axis):
    x = np.asarray(x, np.float32)
    m = x.max(axis=axis, keepdims=True)
    e = np.exp(x - m)
    return e / e.sum(axis=axis, keepdims=True)


def kernel(x, basis, read_coeffs, write_coeffs, op_logits, op_weights,
           op_biases, out_scale):
    global LAST_RESULT
    import ml_dtypes
    from concourse.bass_utils import run_bass_kernel_spmd
    bf16 = ml_dtypes.bfloat16

    x = np.asarray(x, np.float32)
    basis = np.asarray(basis, np.float32)
    read_coeffs = np.asarray(read_coeffs, np.float32)
    write_coeffs = np.asarray(write_coeffs, np.float32)
    op_logits = np.asarray(op_logits, np.float32)
    op_weights = np.asarray(op_weights, np.float64)
    op_biases = np.asarray(op_biases, np.float64)
    out_scale = np.float32(out_scale)

    read_w = _softmax(basis @ read_coeffs.T, axis=0)               # (V, C)
    wwT = np.ascontiguousarray((basis @ write_coeffs.T).T) * out_scale  # (C, V)
    w = _softmax(op_logits, axis=0).astype(np.float64)

    # Fold mixture weights into op weights/biases where the nonlinearity
    # allows; merge the two linear ops (identity, neg) into one.
    #   orig i: 0 ident, 1 relu, 2 gelu, 3 square, 4 neg, 5 abs, 6 tanh, 7 sigm
    Wm = [w[0] * op_weights[0] - w[4] * op_weights[4],
          w[1] * op_weights[1],
          op_weights[2],
          np.sqrt(w[3]) * op_weights[3],
          w[5] * op_weights[5],
          op_weights[6],
          op_weights[7]]
    bm = [w[0] * op_biases[0] - w[4] * op_biases[4],
          w[1] * op_biases[1],
          op_biases[2],
          np.sqrt(w[3]) * op_biases[3],
          w[5] * op_biases[5],
          op_biases[6],
          op_biases[7]]
    post = [1.0, 1.0, float(w[2]), 1.0, 1.0, float(w[6]), float(w[7])]

    S_r = float(2 ** int(np.floor(np.log2(300.0 / read_w.max()))))
    S_o = 64.0
    key = tuple(post) + (float(w[0]), float(w[4]), S_r, S_o)
    if key not in _CACHE:
        _CACHE[key] = _build(post, 1.0 / S_r, 1.0 / S_o)
    nc = _CACHE[key]

    opw_eff = np.stack(Wm).astype(np.float32)          # (NK, C, C)
    opb_eff = np.stack(bm).astype(np.float32).T        # (C, NK)

    # rw: (V, C) -> [p, vt, c];  opw: (NK, C, C) -> [p, k, c]
    rwH = np.ascontiguousarray(
        read_w.reshape(NV, 128, C).transpose(1, 0, 2)).reshape(128, NV * C)
    opwH = np.ascontiguousarray(
        opw_eff.transpose(1, 0, 2)).reshape(C, NK * C)

    S = 1024.0
    shared = {
        "rw": (rwH * S_r).astype(ml_dtypes.float8_e4m3),
        "wwT": (wwT * S).astype(ml_dtypes.float8_e5m2),
        "opw": (opwH * S_o).astype(ml_dtypes.float8_e4m3),
        "opb": np.ascontiguousarray(opb_eff),
    }
    # x[b] (T, V): per chunk c a [p, vt, t] slab, concatenated
    x8 = x.astype(ml_dtypes.float8_e4m3)
    toff = [sum(CHUNKS[:i]) for i in range(NCH)]
    in_maps = []
    for b in range(B):
        slabs = []
        for c, Tc in enumerate(CHUNKS):
            xc = x8[b][toff[c]:toff[c] + Tc]           # (Tc, V)
            # (Tc, V) -> [p, vt, t] with v = vt*128 + p
            slabs.append(np.ascontiguousarray(
                xc.reshape(Tc, NV, 128).transpose(2, 1, 0)).reshape(128, -1))
        m = dict(shared)
        m["xh"] = np.ascontiguousarray(np.concatenate(slabs, axis=1))
        in_maps.append(m)

    res = run_bass_kernel_spmd(nc, in_maps, core_ids=list(range(NCORES)))
    LAST_RESULT = res
    out = np.empty((B, T, V), np.float32)
    for b in range(B):
        d = np.asarray(res.results[b]["out"], np.float32) / S
        negsum = np.asarray(res.results[b]["asum"], np.float32)  # (C, NCH)
        for c, Tc in enumerate(CHUNKS):
            mean = -negsum[:, c] / Tc                            # (C,)
            base = mean @ wwT                                    # (V,)
            out[b, toff[c]:toff[c] + Tc] = d[toff[c]:toff[c] + Tc] + base
    return out


# revision 29
# speedup vs baseline: 1.0453x; 1.0254x over previous
"""Trainium2 Bass kernel for the LGP-instruction module (read -> op bank -> write).

Data-parallel over batch: core b computes x[b] (2048, 4096).
Memory-bound problem: all HBM traffic is 8-bit (x fp8e4m3, out fp8e5m2 as a
scaled delta from a per-chunk mean; params bf16).  rel-err budget (2e-2)
covers this comfortably; most input quantization error is damped because the
output norm is dominated by a constant (sigmoid op) component.

Per T-chunk (tapered sizes to shorten pipeline ramp and tail):
  phase 1: values[C, Tc] = sum_vt rw[vt].T @ x_tile[vt]   (fp8 DoubleRow MMs,
           K=256 per MM; rw scaled by S_r into fp8 range, undone in the
           PSUM->SBUF drain)
  phase 2: h_k = W_k.T @ vals -> ACT f_k(h+b_k) -> DVE bf16 accumulate
           (identity and neg ops pre-merged on host: 7 effective ops);
           then subtract the chunk mean (returned to host via `asum`)
  phase 3: out[Tc, V] = dacc.T @ (wwT*S) -> PSUM f32 -> fp8e5m2 drains
           (split DVE/ACT) -> SWDGE stores.  Host adds mean.T @ wwT back.

Emission interleaves next-chunk phase-1 MMs with current-chunk write pairs so
PSUM drains flow continuously and the PE stays HAM-warm.
"""
import sys
import numpy as np

if '/opt/trn_rl_repo' not in sys.path:
    sys.path.insert(0, '/opt/trn_rl_repo')

B, T, V, C, NOPS = 8, 2048, 4096, 128, 8
NCORES = 8
NV = V // 128                      # 32 v-tiles
CHUNKS = [256, 512, 512, 512, 256]
NCH = len(CHUNKS)
assert sum(CHUNKS) == T
NK = 7                             # effective ops after identity+neg merge
ACT_SET = {1, 3, 5, 7, 9, 11, 13, 15}   # 8/16 psum drain pairs go to ACT

_CACHE = {}
LAST_RESULT = None


def _build(post, inv_sr, inv_so):
    from concourse import bass, bacc, tile, mybir
    f32, bf16 = mybir.dt.float32, mybir.dt.bfloat16
    fp8 = mybir.dt.float8e4
    fp8o = mybir.dt.float8e5
    AF = mybir.ActivationFunctionType
    ts = bass.ts
    FUNCS = [AF.Identity, AF.Relu, AF.Gelu, AF.Square,
             AF.Abs, AF.Tanh, AF.Sigmoid]

    nc = bacc.Bacc("TRN2", target_bir_lowering=False, debug=False,
                   num_devices=NCORES)
    xh = nc.dram_tensor("xh", [128, NV * T], fp8, kind="ExternalInput")
    rw = nc.dram_tensor("rw", [128, NV * C], fp8, kind="ExternalInput")
    wwT = nc.dram_tensor("wwT", [C, V], fp8o, kind="ExternalInput")
    opw = nc.dram_tensor("opw", [C, NK * C], fp8, kind="ExternalInput")
    opb = nc.dram_tensor("opb", [C, NK], f32, kind="ExternalInput")
    out = nc.dram_tensor("out", [T, V], fp8o, kind="ExternalOutput")
    asum = nc.dram_tensor("asum", [C, NCH], f32, kind="ExternalOutput")
    out_r = out.ap().rearrange("(r p) v -> p r v", p=128)

    # chunk c covers t in [toff[c], toff[c]+Tc); x for chunk c is a
    # contiguous [128, NV * Tc] slab at element offset NV * toff[c].
    toff = [sum(CHUNKS[:i]) for i in range(NCH)]

    with tile.TileContext(nc) as tc:
        with tc.tile_pool(name="const", bufs=1) as constp, \
             tc.tile_pool(name="xt", bufs=6) as xtp, \
             tc.tile_pool(name="vals_ps", bufs=1, space="PSUM") as vpsp, \
             tc.tile_pool(name="vals_sb", bufs=2) as vsbp, \
             tc.tile_pool(name="h_ps", bufs=1, space="PSUM") as hpsp, \
             tc.tile_pool(name="t_sb", bufs=3) as tp, \
             tc.tile_pool(name="acc", bufs=2) as accp, \
             tc.tile_pool(name="dacc", bufs=2) as daccp, \
             tc.tile_pool(name="mean", bufs=2) as meanp, \
             tc.tile_pool(name="out_ps", bufs=3, space="PSUM") as opsp, \
             tc.tile_pool(name="out_sb", bufs=4) as osbp:

            # rw piece 0 first on the SP ring (first MMs need it); other
            # consts go on the ACT HWDGE ring so x loads aren't behind them.
            rw_t = constp.tile([128, NV, C], fp8)
            nc.sync.dma_start(rw_t[:, ts(0, NV // 4), :],
                              rw.ap()[:, ts(0, NV * C // 4)])
            opb_t = constp.tile([C, NK], f32)
            nc.scalar.dma_start(opb_t[:], opb.ap())
            opw_t = constp.tile([C, NK * C], fp8)
            nc.scalar.dma_start(opw_t[:], opw.ap())
            wwT_t = constp.tile([C, V], fp8o)
            nc.scalar.dma_start(wwT_t[:], wwT.ap())

            asum_t = constp.tile([C, NCH], f32)
            vals_sb = [None] * NCH
            dacc_sb = [None] * NCH

            def phase1(cn):
                # read: accumulate all v-tiles into one psum bank via fp8
                # DoubleRow (2 v-tiles per MM).  Yields after each MM.
                Tc = CHUNKS[cn]
                values = vpsp.tile([128, Tc], f32)
                if cn == 0:
                    blocks = [(0, 4), (4, 4), (8, 8), (16, 16)]
                else:
                    blocks = [(0, 16), (16, 16)]
                for bi, (v0, nvb) in enumerate(blocks):
                    xt = xtp.tile([128, nvb, Tc], fp8)
                    start_el = NV * toff[cn] + v0 * Tc
                    assert start_el % (nvb * Tc) == 0
                    nc.sync.dma_start(
                        xt[:], xh.ap()[:, ts(start_el // (nvb * Tc),
                                             nvb * Tc)])
                    if cn == 0 and bi in (1, 3):
                        # rw piece 1 after the leading x pieces; pieces 2-3
                        # after the last chunk-0 block (their MMs run later)
                        for q in ((1,) if bi == 1 else (2, 3)):
                            nc.sync.dma_start(
                                rw_t[:, ts(q, NV // 4), :],
                                rw.ap()[:, ts(q, NV * C // 4)])
                    for j in range(0, nvb, 2):
                        vt = v0 + j
                        nc.tensor.matmul(values[:], rw_t[:, ts(vt // 2, 2), :],
                                         xt[:, ts(j // 2, 2), :],
                                         start=(vt == 0), stop=(vt == NV - 2),
                                         perf_mode=mybir.MatmulPerfMode.DoubleRow)
                        yield
                vals = vsbp.tile([128, Tc], bf16)
                nc.vector.tensor_scalar_mul(vals[:], values[:], inv_sr)
                vals_sb[cn] = vals

            def opbank(cn):
                # generator: yields after each op's act so chain steps can be
                # woven between the previous chunk's psum drains
                Tc = CHUNKS[cn]
                vals = vals_sb[cn]
                acc = accp.tile([128, Tc], bf16)
                for k in range(NK):
                    h = hpsp.tile([128, Tc], f32)
                    nc.tensor.matmul(h[:], opw_t[:, ts(k, C)], vals[:],
                                     start=True, stop=True)
                    if k == 0:
                        nc.scalar.activation(acc[:], h[:], FUNCS[0],
                                             bias=opb_t[:, 0:1], scale=inv_so)
                    else:
                        t = tp.tile([128, Tc], bf16)
                        nc.scalar.activation(t[:], h[:], FUNCS[k],
                                             bias=opb_t[:, k:k + 1],
                                             scale=inv_so)
                        nc.vector.scalar_tensor_tensor(
                            acc[:], t[:], post[k], acc[:],
                            op0=mybir.AluOpType.mult, op1=mybir.AluOpType.add)
                    yield
                # subtract the chunk mean (negated sum returned via asum);
                # host adds (-asum/Tc) @ wwT back.
                nc.vector.tensor_reduce(asum_t[:, cn:cn + 1], acc[:],
                                        axis=mybir.AxisListType.X,
                                        op=mybir.AluOpType.add, negate=True)
                negmean = meanp.tile([C, 1], f32)
                nc.vector.tensor_scalar_mul(negmean[:], asum_t[:, cn:cn + 1],
                                            1.0 / Tc)
                dacc = daccp.tile([128, Tc], bf16)
                nc.vector.tensor_scalar_add(dacc[:], acc[:], negmean[:])
                dacc_sb[cn] = dacc

            def write(cn, dacc):
                Tc = CHUNKS[cn]
                nsub = Tc // 128
                row0 = toff[cn] // 128
                for sub2 in range(nsub // 2):
                    osb = osbp.tile([128, 2, V], fp8o)
                    for s in range(2):
                        sub = sub2 * 2 + s
                        for nn2 in range(4):
                            ops2 = opsp.tile([128, 1024], f32)
                            nc.tensor.matmul(ops2[:, 0:512],
                                             dacc[:, ts(sub, 128)],
                                             wwT_t[:, ts(nn2 * 2, 512)],
                                             start=True, stop=True)
                            nc.tensor.matmul(ops2[:, 512:1024],
                                             dacc[:, ts(sub, 128)],
                                             wwT_t[:, ts(nn2 * 2 + 1, 512)],
                                             start=True, stop=True)
                            idx = sub * 4 + nn2
                            if idx in ACT_SET and not (Tc == 512
                                                       and idx == 5):
                                nc.scalar.copy(osb[:, s, ts(nn2, 1024)],
                                               ops2[:])
                            else:
                                nc.vector.tensor_copy(osb[:, s, ts(nn2, 1024)],
                                                      ops2[:])
                            yield
                    seng = nc.sync if cn == NCH - 1 else nc.gpsimd
                    seng.dma_start(
                        out_r[:, ts(row0 // 2 + sub2, 2), :], osb[:])

            def drain(g):
                if g is not None:
                    for _ in g:
                        pass

            def step(g, n):
                if g is None:
                    return None
                for _ in range(n):
                    if next(g, 'done') == 'done':
                        return None
                return g

            # prologue: phase1(0), then weave opbank(0) with phase1(1)
            for _ in phase1(0):
                pass
            g_ob = iter(opbank(0))
            g_p1 = iter(phase1(1))
            while g_ob is not None or g_p1 is not None:
                g_ob = step(g_ob, 1)
                g_p1 = step(g_p1, 2)
            # steady state: write(c) woven with phase1(c+2) and opbank(c+1)
            for c in range(NCH):
                g_p1 = iter(phase1(c + 2)) if c + 2 < NCH else None
                g_ob = iter(opbank(c + 1)) if c + 1 < NCH else None
                if c == NCH - 1:
                    nc.gpsimd.dma_start(asum.ap(), asum_t[:])
                nw = (CHUNKS[c] // 128) * 4
                r1 = max(1, (16 + nw - 1) // nw)
                for _ in write(c, dacc_sb[c]):
                    g_p1 = step(g_p1, r1)
                    g_ob = step(g_ob, 1)
                drain(g_p1)
                drain(g_ob)
    nc.compile()
    return nc


def _softmax(x, axis):
    x = np.asarray(x, np.float32)
    m = x.max(axis=axis, keepdims=True)
    e = np.exp(x - m)
    return e / e.sum(axis=axis, keepdims=True)


def kernel(x, basis, read_coeffs, write_coeffs, op_logits, op_weights,
           op_biases, out_scale):
    global LAST_RESULT
    import ml_dtypes
    from concourse.bass_utils import run_bass_kernel_spmd
    bf16 = ml_dtypes.bfloat16

    x = np.asarray(x, np.float32)
    basis = np.asarray(basis, np.float32)
    read_coeffs = np.asarray(read_coeffs, np.float32)
    write_coeffs = np.asarray(write_coeffs, np.float32)
    op_logits = np.asarray(op_logits, np.float32)
    op_weights = np.asarray(op_weights, np.float64)
    op_biases = np.asarray(op_biases, np.float64)
    out_scale = np.float32(out_scale)

    read_w = _softmax(basis @ read_coeffs.T, axis=0)               # (V, C)
    wwT = np.ascontiguousarray((basis @ write_coeffs.T).T) * out_scale  # (C, V)
    w = _softmax(op_logits, axis=0).astype(np.float64)

    # Fold mixture weights into op weights/biases where the nonlinearity
    # allows; merge the two linear ops (identity, neg) into one.
    #   orig i: 0 ident, 1 relu, 2 gelu, 3 square, 4 neg, 5 abs, 6 tanh, 7 sigm
    Wm = [w[0] * op_weights[0] - w[4] * op_weights[4],
          w[1] * op_weights[1],
          op_weights[2],
          np.sqrt(w[3]) * op_weights[3],
          w[5] * op_weights[5],
          op_weights[6],
          op_weights[7]]
    bm = [w[0] * op_biases[0] - w[4] * op_biases[4],
          w[1] * op_biases[1],
          op_biases[2],
          np.sqrt(w[3]) * op_biases[3],
          w[5] * op_biases[5],
          op_biases[6],
          op_biases[7]]
    post = [1.0, 1.0, float(w[2]), 1.0, 1.0, float(w[6]), float(w[7])]

    S_r = float(2 ** int(np.floor(np.log2(300.0 / read_w.max()))))
    S_o = 64.0
    key = tuple(post) + (float(w[0]), float(w[4]), S_r, S_o)
    if key not in _CACHE:
        _CACHE[key] = _build(post, 1.0 / S_r, 1.0 / S_o)
    nc = _CACHE[key]

    opw_eff = np.stack(Wm).astype(np.float32)          # (NK, C, C)
    opb_eff = np.stack(bm).astype(np.float32).T        # (C, NK)

    # rw: (V, C) -> [p, vt, c];  opw: (NK, C, C) -> [p, k, c]
    rwH = np.ascontiguousarray(
        read_w.reshape(NV, 128, C).transpose(1, 0, 2)).reshape(128, NV * C)
    opwH = np.ascontiguousarray(
        opw_eff.transpose(1, 0, 2)).reshape(C, NK * C)

    S = 1024.0
    shared = {
        "rw": (rwH * S_r).astype(ml_dtypes.float8_e4m3),
        "wwT": (wwT * S).astype(ml_dtypes.float8_e5m2),
        "opw": (opwH * S_o).astype(ml_dtypes.float8_e4m3),
        "opb": np.ascontiguousarray(opb_eff),
    }
    # x[b] (T, V): per chunk c a [p, vt, t] slab, concatenated
    x8 = x.astype(ml_dtypes.float8_e4m3)
    toff = [sum(CHUNKS[:i]) for i in range(NCH)]
    in_maps = []
    for b in range(B):
        slabs = []
        for c, Tc in enumerate(CHUNKS):
            xc = x8[b][toff[c]:toff[c] + Tc]           # (Tc, V)
            # (Tc, V) -> [p, vt, t] with v = vt*128 + p
            slabs.append(np.ascontiguousarray(
                xc.reshape(Tc, NV, 128).transpose(2, 1, 0)).reshape(128, -1))
        m = dict(shared)
        m["xh"] = np.ascontiguousarray(np.concatenate(slabs, axis=1))
        in_maps.append(m)

    res = run_bass_kernel_spmd(nc, in_maps, core_ids=list(range(NCORES)))
    LAST_RESULT = res
    out = np.empty((B, T, V), np.float32)
    for b in range(B):
        d = np.asarray(res.results[b]["out"], np.float32) / S
        negsum = np.asarray(res.results[b]["asum"], np.float32)  # (C, NCH)
        for c, Tc in enumerate(CHUNKS):
            mean = -negsum[:, c] / Tc                            # (C,)
            base = mean @ wwT                                    # (V,)
            out[b, toff[c]:toff[c] + Tc] = d[toff[c]:toff[c] + Tc] + base
    return out
